# revision 11
# baseline (speedup 1.0000x reference)
"""Causal attention (single head, S=4096, d=1024) on 8 TRN2 NeuronCores.

Sharding: core i computes output rows {i + 8m : m in 0..511} (strided
sequence-parallel Q). K/V projections are computed on contiguous chunks
[512i, 512(i+1)) and exchanged with TWO AllGathers: K first (so the
scores pipeline starts ~50us earlier), then V (its latency is hidden
behind the scores/transpose work). Collectives serialize on the ncfw
control plane, so more than two splits only adds floor latency.

All matmuls run in bf16 with f32 PSUM accumulation; softmax statistics
in f32; softmax skips max-subtraction (|q.k|/32 is bounded well inside
f32/bf16 range for any realistic input, and exp of the additive -1e9
mask underflows to exactly 0).

Per 128-row Q chunk b (rows span [1024b, 1024(b+1))), causal attention
needs exactly K[0 : 1024(b+1)] — identical on every core, so one SPMD
program serves all 8 cores; the diagonal-band mask (which depends on the
core index) is passed as a per-core input tensor.

DMA discipline: every dma_start costs ~0.6us of sequencer time plus
~0.6us of the shared HWDGE, so transfers are batched into few large
descriptors and split across the two HWDGE queues (SP=sync,
Activation=scalar) by dependency path: the K-projection feed (xkv) and
K-AG bounce writes ride SP; weight loads and V-AG writes ride
Activation so the K-AllGather trigger never queues behind them.
"""

import numpy as np
import ml_dtypes

import concourse.bass as bass  # noqa: F401  (registers engines)
import concourse.mybir as mybir
from concourse import bacc, tile, masks
from concourse.bass_utils import run_bass_kernel_spmd

SEQ = 4096
D = 1024
N_CORES = 8
CORE_IDS = list(range(N_CORES))
QLOC = SEQ // N_CORES          # 512 q rows per core
NQCH = QLOC // 128             # 4 q chunks of 128 rows
BF16 = mybir.dt.bfloat16
F32 = mybir.dt.float32
MASK_VAL = -1.0e9
SM_SCALE = 1.0 / np.sqrt(np.float32(D))
ACC_BUFS = 4
T_BUFS = 2
O_BUFS = 2


def _emit_compute(nc, tc, pp, dp, cp_tiles, io, rep, variant="full"):
    """Emit one forward pass. `rep` uniquifies collective bounce bufs."""
    ident, mask_sb = cp_tiles
    xq, xkv, wqT, wkT, wvT, out = io

    def dbg_consume(pool, aps, rows):
        """Cheaply consume `aps` (tiny slices) into `out` to defeat DCE."""
        o_dbg = pool.tile([128, 64], F32, tag="dbg", name=f"dbg{rep}_{rows}")
        for idx, ap_ in enumerate(aps[:8]):
            nc.vector.tensor_copy(o_dbg[:, 8 * idx:8 * (idx + 1)], ap_)
        nc.sync.dma_start(out[128 * (rows % 4):128 * (rows % 4) + 128, 0:64],
                          o_dbg[:])

    # DRAM bounce buffers. K block layout per rank: [1024 dout, 512 seq]
    # as (d p) rows; V per rank: [512 seq, 1024 dout] flattened as
    # (c p two) rows of 512 cols.
    k_ag_in = dp.tile([D, QLOC], BF16, tag=f"kagi{rep}", name=f"k_ag_in{rep}")
    k_ag_out = dp.tile([N_CORES * D, QLOC], BF16, addr_space="Shared",
                       tag=f"kago{rep}", name=f"k_ag_out{rep}")
    v_ag_in = dp.tile([D, QLOC], BF16, tag=f"vagi{rep}", name=f"v_ag_in{rep}")
    v_ag_out = dp.tile([N_CORES * D, QLOC], BF16, addr_space="Shared",
                       tag=f"vago{rep}", name=f"v_ag_out{rep}")

    with tc.tile_pool(name="persist", bufs=1) as pers:
        q_sb = pers.tile([128, 8, QLOC], BF16, name="q_sb")     # Q^T [d-chunk, q]
        kT_out = pers.tile([128, 8, QLOC], BF16, name="kT_out")  # own K^T chunk
        v_out = pers.tile([128, 4, D], BF16, name="v_out")       # own V chunk
        sums_all = pers.tile([128, 4, 8], F32, name="sums_all")

        with tc.tile_pool(name="proj", bufs=1) as wp:
            xkv_sb = wp.tile([128, 8, QLOC], BF16, name="xkv_sb")
            xq_sb = wp.tile([128, 8, QLOC], BF16, name="xq_sb")
            wk_sb = wp.tile([128, 8, D], BF16, name="wk_sb")
            wv_sb = wp.tile([128, 8, D], BF16, name="wv_sb")
            wq_sb = wp.tile([128, 8, D], BF16, name="wq_sb")
            xkv_v = xkv.rearrange("(a p) s -> p a s", p=128)
            wk_v = wkT.rearrange("(a p) n -> p a n", p=128)
            wv_v = wvT.rearrange("(a p) n -> p a n", p=128)
            # K-path feeds: xkv on SP, wk/wv on Activation, per-chunk so the
            # first matmuls start as soon as chunk 0 lands.
            for a in range(8):
                nc.sync.dma_start(xkv_sb[:, a, :], xkv_v[:, a, :])
                nc.scalar.dma_start(wk_sb[:, a, :], wk_v[:, a, :])
            for a in range(8):
                nc.scalar.dma_start(wv_sb[:, a, :], wv_v[:, a, :])

            # --- K^T chunk = Wk @ x_chunk^T : [1024 dout, 512 seq]
            for do in range(8):
                ps = pp.tile([128, QLOC], F32, tag="acc", bufs=ACC_BUFS,
                             name=f"ps_k{do}")
                for di in range(8):
                    nc.tensor.matmul(
                        ps[:], wk_sb[:, di, 128 * do:128 * (do + 1)],
                        xkv_sb[:, di, :], start=(di == 0), stop=(di == 7),
                    )
                nc.vector.tensor_copy(kT_out[:, do, :], ps[:])
                if variant != "proj":
                    # stream each d-chunk to the AG input as it completes
                    nc.sync.dma_start(k_ag_in[128 * do:128 * (do + 1), :],
                                      kT_out[:, do, :])
            # xq/wq ride SP behind the k_ag writes; Q proj only needs them
            # once the K-AllGather is already in flight.
            nc.sync.dma_start(xq_sb[:], xq.rearrange("(a p) s -> p a s", p=128))
            nc.sync.dma_start(wq_sb[:], wqT.rearrange("(a p) n -> p a n", p=128))
            if variant != "proj":
                nc.gpsimd.collective_compute(
                    "AllGather", mybir.AluOpType.bypass,
                    ins=[k_ag_in.opt()], outs=[k_ag_out.opt()],
                    replica_groups=[CORE_IDS],
                )

            # --- V chunk = x_chunk @ Wv^T : [512 seq, 1024 dout]
            for c in range(4):
                for h in range(2):
                    ps = pp.tile([128, 512], F32, tag="acc", bufs=ACC_BUFS,
                                 name=f"ps_v{c}_{h}")
                    for di in range(8):
                        nc.tensor.matmul(
                            ps[:], xkv_sb[:, di, 128 * c:128 * (c + 1)],
                            wv_sb[:, di, 512 * h:512 * (h + 1)],
                            start=(di == 0), stop=(di == 7),
                        )
                    nc.vector.tensor_copy(v_out[:, c, 512 * h:512 * (h + 1)],
                                          ps[:])
            if variant != "proj":
                nc.scalar.dma_start(
                    v_ag_in.rearrange("(c p two) s -> p c (two s)",
                                      p=128, two=2),
                    v_out[:])
                nc.gpsimd.collective_compute(
                    "AllGather", mybir.AluOpType.bypass,
                    ins=[v_ag_in.opt()], outs=[v_ag_out.opt()],
                    replica_groups=[CORE_IDS],
                )

            # --- Q^T (strided rows) = Wq @ x_q^T
            for do in range(8):
                ps = pp.tile([128, QLOC], F32, tag="acc", bufs=ACC_BUFS,
                             name=f"ps_q{do}")
                for di in range(8):
                    nc.tensor.matmul(
                        ps[:], wq_sb[:, di, 128 * do:128 * (do + 1)],
                        xq_sb[:, di, :], start=(di == 0), stop=(di == 7),
                    )
                nc.vector.tensor_copy(q_sb[:, do, :], ps[:])

        if variant == "proj":
            dbg_consume(pers, [kT_out[:, 0, 0:8], v_out[:, 0, 0:8],
                               q_sb[:, 0, 0:8]], 0)
            return

        with (
            tc.tile_pool(name="kv", bufs=1) as kvp,
            tc.tile_pool(name="attn", bufs=2) as ap,
        ):
            # Gathered K^T / V land in rank-PAIR tiles (pair j = ranks
            # 2j,2j+1 = exactly the kb range increment per q chunk), one
            # DMA each, alternating queues. K pairs load before V pairs:
            # a V load blocked on the V-AllGather must not sit ahead of K
            # loads in either queue.
            k_pair = [kvp.tile([128, 16, QLOC], BF16, tag=f"k{j}",
                               name=f"k_sb{j}") for j in range(4)]
            v_pair = [kvp.tile([128, 8, D], BF16, tag=f"v{j}",
                               name=f"v_sb{j}") for j in range(4)]
            for j in range(4):
                ksrc = k_ag_out[2048 * j:2048 * (j + 1), :].rearrange(
                    "(r d p) s -> p (r d) s", p=128, r=2)
                eng = nc.sync if j % 2 == 0 else nc.scalar
                eng.dma_start(k_pair[j][:], ksrc[:])
            for j in range(4):
                vsrc = v_ag_out[2048 * j:2048 * (j + 1), :].rearrange(
                    "(r c p two) s -> p (r c) (two s)", p=128, two=2, r=2)
                eng = nc.sync if j % 2 == 0 else nc.scalar
                eng.dma_start(v_pair[j][:], vsrc[:])

            if variant == "projag":
                dbg_consume(ap, [k_pair[j][:, 0, 0:8] for j in range(4)]
                            + [v_pair[j][:, 0, 0:8] for j in range(4)], 0)
                dbg_consume(ap, [q_sb[:, 0, 0:8]], 1)
                return

            for b in range(NQCH):
                nkb = 2 * (b + 1)          # number of 512-wide k blocks
                klen = 512 * nkb
                a_sb = ap.tile([128, klen], BF16, tag=f"A{b}", bufs=1,
                               name=f"a_sb{b}")
                at_sb = ap.tile([128, klen], BF16, tag=f"AT{b}", bufs=1,
                                name=f"at_sb{b}")
                sums = sums_all[:, b, :]

                # scores + exp (no max subtraction: |q.k|/32 is small)
                for kb in range(nkb):
                    ps_s = pp.tile([128, 512], F32, tag="acc", bufs=ACC_BUFS,
                                   name=f"ps_s{b}_{kb}")
                    for di in range(8):
                        nc.tensor.matmul(
                            ps_s[:], q_sb[:, di, 128 * b:128 * (b + 1)],
                            k_pair[kb // 2][:, 8 * (kb % 2) + di, :],
                            start=(di == 0), stop=(di == 7),
                        )
                    if kb >= 2 * b:  # diagonal band: apply causal mask
                        j0 = 512 * (kb - 2 * b)
                        nc.vector.tensor_add(
                            ps_s[:], ps_s[:], mask_sb[:, j0:j0 + 512]
                        )
                    nc.scalar.activation(
                        a_sb[:, 512 * kb:512 * (kb + 1)], ps_s[:],
                        mybir.ActivationFunctionType.Exp,
                        scale=float(SM_SCALE),
                        accum_out=sums[:, kb:kb + 1],
                    )

                # transpose A in 128x128 tiles (PE) -> A^T for the AV matmul
                for kb in range(nkb):
                    ps_t = pp.tile([128, 512], BF16, tag="t", bufs=T_BUFS,
                                   name=f"ps_t{b}_{kb}")
                    for cc in range(4):
                        nc.tensor.transpose(
                            ps_t[:, 128 * cc:128 * (cc + 1)],
                            a_sb[:, 512 * kb + 128 * cc:512 * kb + 128 * (cc + 1)],
                            ident[:],
                        )
                    nc.vector.tensor_copy(
                        at_sb[:, 512 * kb:512 * (kb + 1)], ps_t[:]
                    )

                # row-sum reciprocal
                stot = ap.tile([128, 1], F32, tag="stot", name=f"stot{b}")
                rinv = ap.tile([128, 1], F32, tag="rinv", name=f"rinv{b}")
                nc.vector.reduce_sum(
                    out=stot[:], in_=sums[:, 0:nkb], axis=mybir.AxisListType.X
                )
                nc.vector.reciprocal(rinv[:], stot[:])

                # O = A @ V, then normalize rows by 1/sum
                o_sb = ap.tile([128, D], F32, tag="o", bufs=O_BUFS,
                               name=f"o_sb{b}")
                nkc = klen // 128
                for h in range(2):
                    ps_o = pp.tile([128, 512], F32, tag="o", name=f"ps_o{b}_{h}")
                    for kc in range(nkc):
                        r = kc // 4
                        nc.tensor.matmul(
                            ps_o[:], at_sb[:, 128 * kc:128 * (kc + 1)],
                            v_pair[r // 2][:, 4 * (r % 2) + kc % 4,
                                           512 * h:512 * (h + 1)],
                            start=(kc == 0), stop=(kc == nkc - 1),
                        )
                    nc.vector.tensor_scalar_mul(
                        o_sb[:, 512 * h:512 * (h + 1)], ps_o[:], rinv[:]
                    )
                eng = nc.sync if b % 2 == 0 else nc.scalar
                eng.dma_start(out[128 * b:128 * (b + 1), :], o_sb[:])


def build_nc(reps=1, variant="full"):
    nc = bacc.Bacc("TRN2", target_bir_lowering=False)

    xq = nc.dram_tensor("xq", [D, QLOC], BF16, kind="ExternalInput")
    xkv = nc.dram_tensor("xkv", [D, QLOC], BF16, kind="ExternalInput")
    wqT = nc.dram_tensor("wqT", [D, D], BF16, kind="ExternalInput")
    wkT = nc.dram_tensor("wkT", [D, D], BF16, kind="ExternalInput")
    wvT = nc.dram_tensor("wvT", [D, D], BF16, kind="ExternalInput")
    mask_in = nc.dram_tensor("mask", [128, 1024], F32, kind="ExternalInput")
    out = nc.dram_tensor("out", [QLOC, D], F32, kind="ExternalOutput")
    io = (xq, xkv, wqT, wkT, wvT, out)

    with tile.TileContext(nc) as tc:
        with (
            tc.tile_pool(name="const", bufs=1) as cp,
            tc.tile_pool(name="psum", bufs=2, space="PSUM") as pp,
            tc.tile_pool(name="dram", bufs=1, space="DRAM") as dp,
        ):
            ident = cp.tile([128, 128], BF16, name="ident")
            masks.make_identity(nc, ident[:])
            mask_sb = cp.tile([128, 1024], F32, name="mask_sb")
            nc.sync.dma_start(mask_sb[:], mask_in[:])
            for rep in range(reps):
                if rep > 0:
                    # serialize reps so the R-slope measures single-shot latency
                    tc.strict_bb_all_engine_barrier()
                _emit_compute(nc, tc, pp, dp, (ident, mask_sb), io, rep, variant)

    nc.compile()
    return nc


_NC_CACHE = None


def _get_nc():
    global _NC_CACHE
    if _NC_CACHE is None:
        _NC_CACHE = build_nc()
    return _NC_CACHE


def make_in_maps(x, Wq, Wk, Wv):
    x = np.asarray(x, dtype=np.float32)
    Wq = np.asarray(Wq, dtype=np.float32)
    Wk = np.asarray(Wk, dtype=np.float32)
    Wv = np.asarray(Wv, dtype=np.float32)

    bf = ml_dtypes.bfloat16
    xT = np.ascontiguousarray(x.T).astype(bf)          # [D, SEQ]
    wqT = np.ascontiguousarray(Wq.T).astype(bf)        # [D, D] (d_in major)
    wkT = np.ascontiguousarray(Wk.T).astype(bf)
    wvT = np.ascontiguousarray(Wv.T).astype(bf)

    p = np.arange(128)[:, None]
    j = np.arange(1024)[None, :]
    in_maps = []
    for i in CORE_IDS:
        mask_i = np.where(j <= 8 * p + i, 0.0, MASK_VAL).astype(np.float32)
        in_maps.append({
            "xq": np.ascontiguousarray(xT[:, i::N_CORES]),
            "xkv": np.ascontiguousarray(xT[:, QLOC * i:QLOC * (i + 1)]),
            "wqT": wqT, "wkT": wkT, "wvT": wvT,
            "mask": mask_i,
        })
    return in_maps


def assemble(results):
    out = np.empty((SEQ, D), dtype=np.float32)
    for i in CORE_IDS:
        out[i::N_CORES] = results[i]["out"]
    return out


def kernel(x, Wq, Wk, Wv):
    nc = _get_nc()
    in_maps = make_in_maps(x, Wq, Wk, Wv)
    res = run_bass_kernel_spmd(nc, in_maps, core_ids=CORE_IDS)
    return assemble(res.results)


if __name__ == "__main__":
    rng = np.random.RandomState(0)
    x = rng.randn(SEQ, D).astype(np.float32)
    s = 1.0 / np.sqrt(D)
    Wq = (rng.randn(D, D) * s).astype(np.float32)
    Wk = (rng.randn(D, D) * s).astype(np.float32)
    Wv = (rng.randn(D, D) * s).astype(np.float32)
    out = kernel(x, Wq, Wk, Wv)
    print("kernel ran, out shape", out.shape, "mean", out.mean())


# revision 12
# speedup vs baseline: 1.0311x; 1.0311x over previous
"""Causal attention (single head, S=4096, d=1024) on 8 TRN2 NeuronCores.

Sharding: core i computes output rows {i + 8m : m in 0..511} (strided
sequence-parallel Q). K/V projections are computed on contiguous chunks
[512i, 512(i+1)) and exchanged with TWO AllGathers: K first (so the
scores pipeline starts ~50us earlier), then V (its latency is hidden
behind the scores/transpose work). Collectives serialize on the ncfw
control plane, so more than two splits only adds floor latency.

All matmuls run in bf16 with f32 PSUM accumulation; softmax statistics
in f32; softmax skips max-subtraction (|q.k|/32 is bounded well inside
f32/bf16 range for any realistic input, and exp of the additive -1e9
mask underflows to exactly 0).

Per 128-row Q chunk b (rows span [1024b, 1024(b+1))), causal attention
needs exactly K[0 : 1024(b+1)] — identical on every core, so one SPMD
program serves all 8 cores; the diagonal-band mask (which depends on the
core index) is passed as a per-core input tensor.

DMA discipline: every dma_start costs ~0.6us of sequencer time plus
~0.6us of the shared HWDGE, so transfers are batched into few large
descriptors and split across the two HWDGE queues (SP=sync,
Activation=scalar) by dependency path: the K-projection feed (xkv) and
K-AG bounce writes ride SP; weight loads and V-AG writes ride
Activation so the K-AllGather trigger never queues behind them.
"""

import numpy as np
import ml_dtypes

import concourse.bass as bass  # noqa: F401  (registers engines)
import concourse.mybir as mybir
from concourse import bacc, tile, masks
from concourse.bass_utils import run_bass_kernel_spmd

SEQ = 4096
D = 1024
N_CORES = 8
CORE_IDS = list(range(N_CORES))
QLOC = SEQ // N_CORES          # 512 q rows per core
NQCH = QLOC // 128             # 4 q chunks of 128 rows
BF16 = mybir.dt.bfloat16
F32 = mybir.dt.float32
MASK_VAL = -1.0e9
SM_SCALE = 1.0 / np.sqrt(np.float32(D))
ACC_BUFS = 4
T_BUFS = 2
O_BUFS = 2


def _emit_compute(nc, tc, pp, dp, cp_tiles, io, rep, variant="full"):
    """Emit one forward pass. `rep` uniquifies collective bounce bufs."""
    ident, mask_sb = cp_tiles
    xq, xkv, wqT, wkT, wvT, out = io

    def dbg_consume(pool, aps, rows):
        """Cheaply consume `aps` (tiny slices) into `out` to defeat DCE."""
        o_dbg = pool.tile([128, 64], F32, tag="dbg", name=f"dbg{rep}_{rows}")
        for idx, ap_ in enumerate(aps[:8]):
            nc.vector.tensor_copy(o_dbg[:, 8 * idx:8 * (idx + 1)], ap_)
        nc.sync.dma_start(out[128 * (rows % 4):128 * (rows % 4) + 128, 0:64],
                          o_dbg[:])

    # DRAM bounce buffers. K block layout per rank: [1024 dout, 512 seq]
    # as (d p) rows; V per rank: [512 seq, 1024 dout] flattened as
    # (c p two) rows of 512 cols.
    k_ag_in = dp.tile([D, QLOC], BF16, tag=f"kagi{rep}", name=f"k_ag_in{rep}")
    k_ag_out = dp.tile([N_CORES * D, QLOC], BF16, addr_space="Shared",
                       tag=f"kago{rep}", name=f"k_ag_out{rep}")
    v_ag_in = dp.tile([D, QLOC], BF16, tag=f"vagi{rep}", name=f"v_ag_in{rep}")
    v_ag_out = dp.tile([N_CORES * D, QLOC], BF16, addr_space="Shared",
                       tag=f"vago{rep}", name=f"v_ag_out{rep}")

    with tc.tile_pool(name="persist", bufs=1) as pers:
        q_sb = pers.tile([128, 8, QLOC], BF16, name="q_sb")     # Q^T [d-chunk, q]
        kT_out = pers.tile([128, 8, QLOC], BF16, name="kT_out")  # own K^T chunk
        v_out = pers.tile([128, 4, D], BF16, name="v_out")       # own V chunk
        sums_all = pers.tile([128, 4, 8], F32, name="sums_all")

        with tc.tile_pool(name="proj", bufs=1) as wp:
            xkv_sb = wp.tile([128, 8, QLOC], BF16, name="xkv_sb")
            xq_sb = wp.tile([128, 8, QLOC], BF16, name="xq_sb")
            wk_sb = wp.tile([128, 8, D], BF16, name="wk_sb")
            wv_sb = wp.tile([128, 8, D], BF16, name="wv_sb")
            wq_sb = wp.tile([128, 8, D], BF16, name="wq_sb")
            xkv_v = xkv.rearrange("(a p) s -> p a s", p=128)
            wk_v = wkT.rearrange("(a p) n -> p a n", p=128)
            wv_v = wvT.rearrange("(a p) n -> p a n", p=128)
            # K-path feeds: xkv on SP, wk on Activation, halved so the
            # first di-group's matmuls start at the half-load mark.
            nc.sync.dma_start(xkv_sb[:, 0:4, :], xkv_v[:, 0:4, :])
            nc.sync.dma_start(xkv_sb[:, 4:8, :], xkv_v[:, 4:8, :])
            nc.scalar.dma_start(wk_sb[:, 0:4, :], wk_v[:, 0:4, :])
            nc.scalar.dma_start(wk_sb[:, 4:8, :], wk_v[:, 4:8, :])
            nc.scalar.dma_start(wv_sb[:], wv_v[:])

            # --- K^T chunk = Wk @ x_chunk^T : [1024 dout, 512 seq]
            # di-major inside each 4-wide do-group: all four psums finish
            # right after the last feed chunk lands → the K-AG bounce write
            # (and with it the AllGather trigger) happens ~8us earlier than
            # do-major order would allow.
            for g in range(2):
                pss = [pp.tile([128, QLOC], F32, tag="acc", bufs=ACC_BUFS,
                               name=f"ps_k{4 * g + j}") for j in range(4)]
                for di in range(8):
                    for j in range(4):
                        do = 4 * g + j
                        nc.tensor.matmul(
                            pss[j][:], wk_sb[:, di, 128 * do:128 * (do + 1)],
                            xkv_sb[:, di, :], start=(di == 0), stop=(di == 7),
                        )
                for j in range(4):
                    nc.vector.tensor_copy(kT_out[:, 4 * g + j, :], pss[j][:])
                if variant != "proj":
                    nc.sync.dma_start(
                        k_ag_in[512 * g:512 * (g + 1), :].rearrange(
                            "(d p) s -> p d s", p=128),
                        kT_out[:, 4 * g:4 * (g + 1), :])
            # xq/wq ride SP behind the k_ag writes; Q proj only needs them
            # once the K-AllGather is already in flight.
            nc.sync.dma_start(xq_sb[:], xq.rearrange("(a p) s -> p a s", p=128))
            nc.sync.dma_start(wq_sb[:], wqT.rearrange("(a p) n -> p a n", p=128))
            if variant != "proj":
                nc.gpsimd.collective_compute(
                    "AllGather", mybir.AluOpType.bypass,
                    ins=[k_ag_in.opt()], outs=[k_ag_out.opt()],
                    replica_groups=[CORE_IDS],
                )

            # --- V chunk = x_chunk @ Wv^T : [512 seq, 1024 dout]
            for c in range(4):
                for h in range(2):
                    ps = pp.tile([128, 512], F32, tag="acc", bufs=ACC_BUFS,
                                 name=f"ps_v{c}_{h}")
                    for di in range(8):
                        nc.tensor.matmul(
                            ps[:], xkv_sb[:, di, 128 * c:128 * (c + 1)],
                            wv_sb[:, di, 512 * h:512 * (h + 1)],
                            start=(di == 0), stop=(di == 7),
                        )
                    nc.vector.tensor_copy(v_out[:, c, 512 * h:512 * (h + 1)],
                                          ps[:])
            if variant != "proj":
                nc.scalar.dma_start(
                    v_ag_in.rearrange("(c p two) s -> p c (two s)",
                                      p=128, two=2),
                    v_out[:])
                nc.gpsimd.collective_compute(
                    "AllGather", mybir.AluOpType.bypass,
                    ins=[v_ag_in.opt()], outs=[v_ag_out.opt()],
                    replica_groups=[CORE_IDS],
                )

            # --- Q^T (strided rows) = Wq @ x_q^T
            for do in range(8):
                ps = pp.tile([128, QLOC], F32, tag="acc", bufs=ACC_BUFS,
                             name=f"ps_q{do}")
                for di in range(8):
                    nc.tensor.matmul(
                        ps[:], wq_sb[:, di, 128 * do:128 * (do + 1)],
                        xq_sb[:, di, :], start=(di == 0), stop=(di == 7),
                    )
                nc.vector.tensor_copy(q_sb[:, do, :], ps[:])

        if variant == "proj":
            dbg_consume(pers, [kT_out[:, 0, 0:8], v_out[:, 0, 0:8],
                               q_sb[:, 0, 0:8]], 0)
            return

        with (
            tc.tile_pool(name="kv", bufs=1) as kvp,
            tc.tile_pool(name="attn", bufs=2) as ap,
        ):
            # Gathered K^T / V land in rank-PAIR tiles (pair j = ranks
            # 2j,2j+1 = exactly the kb range increment per q chunk), one
            # DMA each, alternating queues. K pairs load before V pairs:
            # a V load blocked on the V-AllGather must not sit ahead of K
            # loads in either queue.
            k_pair = [kvp.tile([128, 16, QLOC], BF16, tag=f"k{j}",
                               name=f"k_sb{j}") for j in range(4)]
            v_pair = [kvp.tile([128, 8, D], BF16, tag=f"v{j}",
                               name=f"v_sb{j}") for j in range(4)]
            for j in range(4):
                ksrc = k_ag_out[2048 * j:2048 * (j + 1), :].rearrange(
                    "(r d p) s -> p (r d) s", p=128, r=2)
                eng = nc.sync if j % 2 == 0 else nc.scalar
                eng.dma_start(k_pair[j][:], ksrc[:])
            for j in range(4):
                vsrc = v_ag_out[2048 * j:2048 * (j + 1), :].rearrange(
                    "(r c p two) s -> p (r c) (two s)", p=128, two=2, r=2)
                eng = nc.sync if j % 2 == 0 else nc.scalar
                eng.dma_start(v_pair[j][:], vsrc[:])

            if variant == "projag":
                dbg_consume(ap, [k_pair[j][:, 0, 0:8] for j in range(4)]
                            + [v_pair[j][:, 0, 0:8] for j in range(4)], 0)
                dbg_consume(ap, [q_sb[:, 0, 0:8]], 1)
                return

            for b in range(NQCH):
                nkb = 2 * (b + 1)          # number of 512-wide k blocks
                klen = 512 * nkb
                a_sb = ap.tile([128, klen], BF16, tag=f"A{b}", bufs=1,
                               name=f"a_sb{b}")
                at_sb = ap.tile([128, klen], BF16, tag=f"AT{b}", bufs=1,
                                name=f"at_sb{b}")
                sums = sums_all[:, b, :]

                # scores + exp (no max subtraction: |q.k|/32 is small)
                for kb in range(nkb):
                    ps_s = pp.tile([128, 512], F32, tag="acc", bufs=ACC_BUFS,
                                   name=f"ps_s{b}_{kb}")
                    for di in range(8):
                        nc.tensor.matmul(
                            ps_s[:], q_sb[:, di, 128 * b:128 * (b + 1)],
                            k_pair[kb // 2][:, 8 * (kb % 2) + di, :],
                            start=(di == 0), stop=(di == 7),
                        )
                    if kb >= 2 * b:  # diagonal band: apply causal mask
                        j0 = 512 * (kb - 2 * b)
                        nc.vector.tensor_add(
                            ps_s[:], ps_s[:], mask_sb[:, j0:j0 + 512]
                        )
                    nc.scalar.activation(
                        a_sb[:, 512 * kb:512 * (kb + 1)], ps_s[:],
                        mybir.ActivationFunctionType.Exp,
                        scale=float(SM_SCALE),
                        accum_out=sums[:, kb:kb + 1],
                    )

                # transpose A in 128x128 tiles (PE) -> A^T for the AV matmul
                for kb in range(nkb):
                    ps_t = pp.tile([128, 512], BF16, tag="t", bufs=T_BUFS,
                                   name=f"ps_t{b}_{kb}")
                    for cc in range(4):
                        nc.tensor.transpose(
                            ps_t[:, 128 * cc:128 * (cc + 1)],
                            a_sb[:, 512 * kb + 128 * cc:512 * kb + 128 * (cc + 1)],
                            ident[:],
                        )
                    nc.vector.tensor_copy(
                        at_sb[:, 512 * kb:512 * (kb + 1)], ps_t[:]
                    )

                # row-sum reciprocal
                stot = ap.tile([128, 1], F32, tag="stot", name=f"stot{b}")
                rinv = ap.tile([128, 1], F32, tag="rinv", name=f"rinv{b}")
                nc.vector.reduce_sum(
                    out=stot[:], in_=sums[:, 0:nkb], axis=mybir.AxisListType.X
                )
                nc.vector.reciprocal(rinv[:], stot[:])

                # O = A @ V, then normalize rows by 1/sum
                o_sb = ap.tile([128, D], F32, tag="o", bufs=O_BUFS,
                               name=f"o_sb{b}")
                nkc = klen // 128
                for h in range(2):
                    ps_o = pp.tile([128, 512], F32, tag="o", name=f"ps_o{b}_{h}")
                    for kc in range(nkc):
                        r = kc // 4
                        nc.tensor.matmul(
                            ps_o[:], at_sb[:, 128 * kc:128 * (kc + 1)],
                            v_pair[r // 2][:, 4 * (r % 2) + kc % 4,
                                           512 * h:512 * (h + 1)],
                            start=(kc == 0), stop=(kc == nkc - 1),
                        )
                    nc.vector.tensor_scalar_mul(
                        o_sb[:, 512 * h:512 * (h + 1)], ps_o[:], rinv[:]
                    )
                eng = nc.sync if b % 2 == 0 else nc.scalar
                eng.dma_start(out[128 * b:128 * (b + 1), :], o_sb[:])


def build_nc(reps=1, variant="full"):
    nc = bacc.Bacc("TRN2", target_bir_lowering=False)

    xq = nc.dram_tensor("xq", [D, QLOC], BF16, kind="ExternalInput")
    xkv = nc.dram_tensor("xkv", [D, QLOC], BF16, kind="ExternalInput")
    wqT = nc.dram_tensor("wqT", [D, D], BF16, kind="ExternalInput")
    wkT = nc.dram_tensor("wkT", [D, D], BF16, kind="ExternalInput")
    wvT = nc.dram_tensor("wvT", [D, D], BF16, kind="ExternalInput")
    mask_in = nc.dram_tensor("mask", [128, 1024], F32, kind="ExternalInput")
    out = nc.dram_tensor("out", [QLOC, D], F32, kind="ExternalOutput")
    io = (xq, xkv, wqT, wkT, wvT, out)

    with tile.TileContext(nc) as tc:
        with (
            tc.tile_pool(name="const", bufs=1) as cp,
            tc.tile_pool(name="psum", bufs=2, space="PSUM") as pp,
            tc.tile_pool(name="dram", bufs=1, space="DRAM") as dp,
        ):
            ident = cp.tile([128, 128], BF16, name="ident")
            masks.make_identity(nc, ident[:])
            mask_sb = cp.tile([128, 1024], F32, name="mask_sb")
            nc.sync.dma_start(mask_sb[:], mask_in[:])
            for rep in range(reps):
                if rep > 0:
                    # serialize reps so the R-slope measures single-shot latency
                    tc.strict_bb_all_engine_barrier()
                _emit_compute(nc, tc, pp, dp, (ident, mask_sb), io, rep, variant)

    nc.compile()
    return nc


_NC_CACHE = None


def _get_nc():
    global _NC_CACHE
    if _NC_CACHE is None:
        _NC_CACHE = build_nc()
    return _NC_CACHE


def make_in_maps(x, Wq, Wk, Wv):
    x = np.asarray(x, dtype=np.float32)
    Wq = np.asarray(Wq, dtype=np.float32)
    Wk = np.asarray(Wk, dtype=np.float32)
    Wv = np.asarray(Wv, dtype=np.float32)

    bf = ml_dtypes.bfloat16
    xT = np.ascontiguousarray(x.T).astype(bf)          # [D, SEQ]
    wqT = np.ascontiguousarray(Wq.T).astype(bf)        # [D, D] (d_in major)
    wkT = np.ascontiguousarray(Wk.T).astype(bf)
    wvT = np.ascontiguousarray(Wv.T).astype(bf)

    p = np.arange(128)[:, None]
    j = np.arange(1024)[None, :]
    in_maps = []
    for i in CORE_IDS:
        mask_i = np.where(j <= 8 * p + i, 0.0, MASK_VAL).astype(np.float32)
        in_maps.append({
            "xq": np.ascontiguousarray(xT[:, i::N_CORES]),
            "xkv": np.ascontiguousarray(xT[:, QLOC * i:QLOC * (i + 1)]),
            "wqT": wqT, "wkT": wkT, "wvT": wvT,
            "mask": mask_i,
        })
    return in_maps


def assemble(results):
    out = np.empty((SEQ, D), dtype=np.float32)
    for i in CORE_IDS:
        out[i::N_CORES] = results[i]["out"]
    return out


def kernel(x, Wq, Wk, Wv):
    nc = _get_nc()
    in_maps = make_in_maps(x, Wq, Wk, Wv)
    res = run_bass_kernel_spmd(nc, in_maps, core_ids=CORE_IDS)
    return assemble(res.results)


if __name__ == "__main__":
    rng = np.random.RandomState(0)
    x = rng.randn(SEQ, D).astype(np.float32)
    s = 1.0 / np.sqrt(D)
    Wq = (rng.randn(D, D) * s).astype(np.float32)
    Wk = (rng.randn(D, D) * s).astype(np.float32)
    Wv = (rng.randn(D, D) * s).astype(np.float32)
    out = kernel(x, Wq, Wk, Wv)
    print("kernel ran, out shape", out.shape, "mean", out.mean())


# revision 15
# speedup vs baseline: 1.4851x; 1.4403x over previous
"""Causal attention (single head, S=4096, d=1024) on 8 TRN2 NeuronCores —
collective-free formulation.

Core i computes output rows {i + 8m} (strided sequence-parallel Q). The
K/V AllGathers of the naive sharding are eliminated algebraically: with
K^T = Wk X^T and V = X Wv^T, and the full input X replicated to every
core as a kernel input (full_io),

    S = Q K^T = (Q Wk) X^T      (G := Q Wk is [512, 1024], local)
    O = A V   = (A X) Wv^T      (apply Wv once per core at the end)

so no inter-core communication is needed at all, and per-core matmul
work is unchanged: three [512x1024x1024] projections (Q, G, final Wv)
plus the causal scores/AV contractions. The output is produced
transposed (O^T) to keep the final projection's moving dim 512 wide;
the host assembles with a transpose.

Numerics: bf16 matmuls with f32 PSUM accumulation; softmax statistics
f32; exp skips max-subtraction (|q.k|/32 is bounded well inside bf16
range; exp of the additive -1e9 mask underflows to exactly 0). The
extra bf16 rounding of G adds ~sqrt(2)x score noise vs the direct
Q.K^T — well inside the error budget.
"""

import numpy as np
import ml_dtypes

import concourse.bass as bass  # noqa: F401  (registers engines)
import concourse.mybir as mybir
from concourse import bacc, tile, masks
from concourse.bass_utils import run_bass_kernel_spmd

SEQ = 4096
D = 1024
N_CORES = 8
CORE_IDS = list(range(N_CORES))
QLOC = SEQ // N_CORES          # 512 q rows per core
NQCH = QLOC // 128
OUT_SHAPE = (1024, 512)  # out dram tensor is O^T [D, QLOC]             # 4 q chunks of 128 rows
BF16 = mybir.dt.bfloat16
F32 = mybir.dt.float32
MASK_VAL = -1.0e9
SM_SCALE = 1.0 / np.sqrt(np.float32(D))
ACC_BUFS = 4
T_BUFS = 2
O_BUFS = 2


def _emit_compute(nc, tc, pp, cp_tiles, io, rep, variant="full"):
    ident, mask_sb = cp_tiles
    xq, xt, xn, wqT, wkN, wvT, out = io

    with tc.tile_pool(name="persist", bufs=1) as pers:
        g_sb = pers.tile([128, 8, QLOC], BF16, name="g_sb")    # G^T [din, q]
        axT_sb = pers.tile([128, 8, QLOC], BF16, name="axT_sb")  # (AX)^T
        ot_sb = pers.tile([128, 8, QLOC], F32, name="ot_sb")    # O^T
        sums_all = pers.tile([128, 4, 8], F32, name="sums_all")

        with tc.tile_pool(name="xt", bufs=1) as xtp:
            xt_sb = xtp.tile([128, 8, SEQ], BF16, name="xt_sb")  # X^T d-major
            xt_v = xt.rearrange("(a p) s -> p a s", p=128)
            for a in range(8):
                eng = nc.sync if a % 2 == 0 else nc.scalar
                eng.dma_start(xt_sb[:, a, :], xt_v[:, a, :])

            with tc.tile_pool(name="proj", bufs=1) as wp:
                xq_sb = wp.tile([128, 8, QLOC], BF16, name="xq_sb")
                q_sb = wp.tile([128, 8, QLOC], BF16, name="q_sb")
                wq_sb = wp.tile([128, 8, D], BF16, name="wq_sb")
                wkn_sb = wp.tile([128, 8, D], BF16, name="wkn_sb")
                nc.sync.dma_start(xq_sb[:],
                                  xq.rearrange("(a p) s -> p a s", p=128))
                nc.sync.dma_start(wq_sb[:],
                                  wqT.rearrange("(a p) n -> p a n", p=128))
                nc.scalar.dma_start(wkn_sb[:],
                                    wkN.rearrange("(a p) n -> p a n", p=128))

                # --- Q^T (strided rows) = Wq @ x_q^T : [1024 do, 512 q]
                for do in range(8):
                    ps = pp.tile([128, QLOC], F32, tag="acc", bufs=ACC_BUFS,
                                 name=f"ps_q{do}")
                    for di in range(8):
                        nc.tensor.matmul(
                            ps[:], wq_sb[:, di, 128 * do:128 * (do + 1)],
                            xq_sb[:, di, :], start=(di == 0), stop=(di == 7),
                        )
                    nc.vector.tensor_copy(q_sb[:, do, :], ps[:])

                # --- G^T = Wk^T @ Q^T : [1024 din, 512 q]
                # lhsT = Wk[do, di] slices (row-major Wk input), rhs = Q^T.
                for gi in range(8):
                    ps = pp.tile([128, QLOC], F32, tag="acc", bufs=ACC_BUFS,
                                 name=f"ps_g{gi}")
                    for do in range(8):
                        nc.tensor.matmul(
                            ps[:], wkn_sb[:, do, 128 * gi:128 * (gi + 1)],
                            q_sb[:, do, :], start=(do == 0), stop=(do == 7),
                        )
                    nc.vector.tensor_copy(g_sb[:, gi, :], ps[:])

            if variant == "proj":
                o_dbg = pers.tile([128, 64], F32, tag="dbg", name=f"dbg{rep}")
                nc.vector.tensor_copy(o_dbg[:, 0:8], g_sb[:, 0, 0:8])
                nc.vector.tensor_copy(o_dbg[:, 8:16], xt_sb[:, 0, 0:8])
                nc.sync.dma_start(out[0:128, 0:64], o_dbg[:])
                return

            with (
                tc.tile_pool(name="xn", bufs=1) as xnp,
                tc.tile_pool(name="late", bufs=1) as lp,
                tc.tile_pool(name="attn", bufs=2) as ap,
            ):
                xn_sb = xnp.tile([128, 32, D], BF16, name="xn_sb")  # X seq-major
                xn_v = xn.rearrange("(blk p) d -> p blk d", p=128)
                for g in range(8):
                    eng = nc.sync if g % 2 == 0 else nc.scalar
                    eng.dma_start(xn_sb[:, 4 * g:4 * (g + 1), :],
                                  xn_v[:, 4 * g:4 * (g + 1), :])
                wv_sb = lp.tile([128, 8, D], BF16, name="wv_sb")
                nc.scalar.dma_start(wv_sb[:],
                                    wvT.rearrange("(a p) n -> p a n", p=128))

                for b in range(NQCH):
                    nkb = 2 * (b + 1)          # number of 512-wide k blocks
                    klen = 512 * nkb
                    a_sb = ap.tile([128, SEQ], BF16, tag="A", bufs=2,
                                   name=f"a_sb{b}")
                    at_sb = ap.tile([128, SEQ], BF16, tag="AT", bufs=1,
                                    name=f"at_sb{b}")
                    sums = sums_all[:, b, :]

                    # scores S = G X^T blockwise + exp
                    for kb in range(nkb):
                        ps_s = pp.tile([128, 512], F32, tag="acc",
                                       bufs=ACC_BUFS, name=f"ps_s{b}_{kb}")
                        for di in range(8):
                            nc.tensor.matmul(
                                ps_s[:], g_sb[:, di, 128 * b:128 * (b + 1)],
                                xt_sb[:, di, 512 * kb:512 * (kb + 1)],
                                start=(di == 0), stop=(di == 7),
                            )
                        if kb >= 2 * b:  # diagonal band: causal mask
                            j0 = 512 * (kb - 2 * b)
                            nc.vector.tensor_add(
                                ps_s[:], ps_s[:], mask_sb[:, j0:j0 + 512]
                            )
                        nc.scalar.activation(
                            a_sb[:, 512 * kb:512 * (kb + 1)], ps_s[:],
                            mybir.ActivationFunctionType.Exp,
                            scale=float(SM_SCALE),
                            accum_out=sums[:, kb:kb + 1],
                        )

                    # transpose A (PE) -> A^T for the AX matmul
                    for kb in range(nkb):
                        ps_t = pp.tile([128, 512], BF16, tag="t", bufs=T_BUFS,
                                       name=f"ps_t{b}_{kb}")
                        for cc in range(4):
                            nc.tensor.transpose(
                                ps_t[:, 128 * cc:128 * (cc + 1)],
                                a_sb[:, 512 * kb + 128 * cc:
                                     512 * kb + 128 * (cc + 1)],
                                ident[:],
                            )
                        nc.vector.tensor_copy(
                            at_sb[:, 512 * kb:512 * (kb + 1)], ps_t[:]
                        )

                    stot = ap.tile([128, 1], F32, tag="stot", name=f"stot{b}")
                    rinv = ap.tile([128, 1], F32, tag="rinv", name=f"rinv{b}")
                    nc.vector.reduce_sum(
                        out=stot[:], in_=sums[:, 0:nkb], axis=mybir.AxisListType.X
                    )
                    nc.vector.reciprocal(rinv[:], stot[:])

                    # AX = A @ X rows [0, klen), normalized by 1/rowsum
                    ax_sb = ap.tile([128, D], BF16, tag="ax", bufs=1,
                                    name=f"ax_sb{b}")
                    nkc = klen // 128
                    for h in range(2):
                        ps_o = pp.tile([128, 512], F32, tag="o", bufs=O_BUFS,
                                       name=f"ps_o{b}_{h}")
                        for kc in range(nkc):
                            nc.tensor.matmul(
                                ps_o[:], at_sb[:, 128 * kc:128 * (kc + 1)],
                                xn_sb[:, kc, 512 * h:512 * (h + 1)],
                                start=(kc == 0), stop=(kc == nkc - 1),
                            )
                        nc.vector.tensor_scalar_mul(
                            ax_sb[:, 512 * h:512 * (h + 1)], ps_o[:], rinv[:]
                        )

                    # transpose AX -> (AX)^T column block b
                    for g2 in range(2):
                        ps_t2 = pp.tile([128, 512], BF16, tag="t", bufs=T_BUFS,
                                        name=f"ps_t2{b}_{g2}")
                        for j in range(4):
                            nc.tensor.transpose(
                                ps_t2[:, 128 * j:128 * (j + 1)],
                                ax_sb[:, 512 * g2 + 128 * j:
                                      512 * g2 + 128 * (j + 1)],
                                ident[:],
                            )
                        for j in range(4):
                            nc.vector.tensor_copy(
                                axT_sb[:, 4 * g2 + j, 128 * b:128 * (b + 1)],
                                ps_t2[:, 128 * j:128 * (j + 1)],
                            )

                # --- O^T = Wv (AX)^T : [1024 do, 512 q]
                for do in range(8):
                    ps = pp.tile([128, QLOC], F32, tag="o", bufs=O_BUFS,
                                 name=f"ps_ot{do}")
                    for di in range(8):
                        nc.tensor.matmul(
                            ps[:], wv_sb[:, di, 128 * do:128 * (do + 1)],
                            axT_sb[:, di, :], start=(di == 0), stop=(di == 7),
                        )
                    nc.vector.tensor_copy(ot_sb[:, do, :], ps[:])
                out_v = out.rearrange("(a p) q -> p a q", p=128)
                nc.sync.dma_start(out_v[:, 0:4, :], ot_sb[:, 0:4, :])
                nc.scalar.dma_start(out_v[:, 4:8, :], ot_sb[:, 4:8, :])


def build_nc(reps=1, variant="full"):
    nc = bacc.Bacc("TRN2", target_bir_lowering=False)

    xq = nc.dram_tensor("xq", [D, QLOC], BF16, kind="ExternalInput")
    xt = nc.dram_tensor("xt", [D, SEQ], BF16, kind="ExternalInput")
    xn = nc.dram_tensor("xn", [SEQ, D], BF16, kind="ExternalInput")
    wqT = nc.dram_tensor("wqT", [D, D], BF16, kind="ExternalInput")
    wkN = nc.dram_tensor("wkN", [D, D], BF16, kind="ExternalInput")
    wvT = nc.dram_tensor("wvT", [D, D], BF16, kind="ExternalInput")
    mask_in = nc.dram_tensor("mask", [128, 1024], F32, kind="ExternalInput")
    out = nc.dram_tensor("out", [D, QLOC], F32, kind="ExternalOutput")
    io = (xq, xt, xn, wqT, wkN, wvT, out)

    with tile.TileContext(nc) as tc:
        with (
            tc.tile_pool(name="const", bufs=1) as cp,
            tc.tile_pool(name="psum", bufs=2, space="PSUM") as pp,
        ):
            ident = cp.tile([128, 128], BF16, name="ident")
            masks.make_identity(nc, ident[:])
            mask_sb = cp.tile([128, 1024], F32, name="mask_sb")
            nc.sync.dma_start(mask_sb[:], mask_in[:])
            for rep in range(reps):
                if rep > 0:
                    # serialize reps so the R-slope measures single-shot latency
                    tc.strict_bb_all_engine_barrier()
                _emit_compute(nc, tc, pp, (ident, mask_sb), io, rep, variant)

    nc.compile()
    return nc


def make_in_maps(x, Wq, Wk, Wv):
    x = np.asarray(x, dtype=np.float32)
    Wq = np.asarray(Wq, dtype=np.float32)
    Wk = np.asarray(Wk, dtype=np.float32)
    Wv = np.asarray(Wv, dtype=np.float32)

    bf = ml_dtypes.bfloat16
    xT = np.ascontiguousarray(x.T).astype(bf)          # [D, SEQ]
    xn = np.ascontiguousarray(x).astype(bf)            # [SEQ, D]
    wqT = np.ascontiguousarray(Wq.T).astype(bf)
    wkN = np.ascontiguousarray(Wk).astype(bf)          # row-major [dout, din]
    wvT = np.ascontiguousarray(Wv.T).astype(bf)

    p = np.arange(128)[:, None]
    j = np.arange(1024)[None, :]
    in_maps = []
    for i in CORE_IDS:
        mask_i = np.where(j <= 8 * p + i, 0.0, MASK_VAL).astype(np.float32)
        in_maps.append({
            "xq": np.ascontiguousarray(xT[:, i::N_CORES]),
            "xt": xT, "xn": xn,
            "wqT": wqT, "wkN": wkN, "wvT": wvT,
            "mask": mask_i,
        })
    return in_maps


def assemble(results):
    out = np.empty((SEQ, D), dtype=np.float32)
    for i in CORE_IDS:
        out[i::N_CORES] = results[i]["out"].T
    return out


def kernel(x, Wq, Wk, Wv):
    global _NC_CACHE
    if _NC_CACHE is None:
        _NC_CACHE = build_nc()
    in_maps = make_in_maps(x, Wq, Wk, Wv)
    res = run_bass_kernel_spmd(nc := _NC_CACHE, in_maps, core_ids=CORE_IDS)
    return assemble(res.results)


_NC_CACHE = None
